# revision 50
# baseline (speedup 1.0000x reference)
"""KG scoring kernel: scores[b,e] = W2 . relu(ent[e]@W1t + ph[b]) + b2
over all entities, sharded across 8 TRN2 NeuronCores along the entity axis.

Device computes bf16-approximate FULL scores for its 6250-entity shard;
host re-assembles [B, E], takes approximate top-C, exactly rescores the
candidates in fp32 and returns the exact top-k (so the bf16 screen only
has to keep true winners inside the top-256, margin ~10 sigma).

Per-core dataflow (engine-balanced; ~32 us in CoreSim vs 222 us for the
fp32 M=1-matmul baseline):
  PE   : bf16 GEMM pt = W1t.T @ entT per 512-col chunk -> PSUM [h, e];
         then 784 stationary-u matmuls score[e-tile, b-col] += u.T @ w2
         (output free size 1, so the reduce is ~free on PE; scores land
         entity-major across partitions -> no partition remap needed)
  ACT  : cast-stages all pt chunks PSUM->SBUF bf16 (GPSIMD cannot read
         PSUM on real HW), relu planes for the tail columns (b<6), and
         score evacuation pieces
  Pool : relu planes for an early and a late column piece (SBUF only)
  DVE  : relu planes for ~70% of columns in 4x mode (bf16 all-SBUF,
         0.26 ns/elem), issued in a slanted (batch, range) order so it
         starts as soon as the first chunks are staged; also covers the
         tail columns for b in {6,7} that ACT would reach too late
  DMA  : ent shard in (bf16), score pieces out; queues overlap by engine

u[b] = relu(pt + qb[b]) tiles are a 6-deep ring; the reduce is b-outer
so u[b] frees after its 98 matmuls. One PSUM accumulation group spans
the whole scores bank (zero region = whole 2KB bank); each column's
first matmul overwrites via the pending-zero mechanism, its h1 partner
accumulates.
"""

import numpy as np
import ml_dtypes

import concourse.bass as bass
import concourse.bacc as bacc
import concourse.tile as tile
from concourse import mybir
from concourse import bass_utils

B = 8            # batch
E = 50000        # entities
D = 128          # embedding dim
H = 256          # hidden dim
NCORES = 8
E_LOC = E // NCORES          # 6250 entities per core
NT = 49                      # e-tiles of 128 per core
E_PAD = NT * 128             # 6272
# GEMM chunk widths: two small chunks first so DVE's first relu range is
# staged early, then 512-wide, 128 tail
CHUNK_W = [256, 256] + [512] * 10 + [128, 512]
CHUNK_OFF = [sum(CHUNK_W[:i]) for i in range(len(CHUNK_W))]
NCHUNK = len(CHUNK_W)        # 14; the last 512-chunk stays pinned in PSUM
                             # and ACT relus it directly (never staged)

# relu column ownership (cols; need not align to e-tiles — the reduce
# just depends on two producers at a boundary). GPSIMD cannot read PSUM
# on real HW, so ACT does ALL PSUM->SBUF staging + evac; Pool gets pure
# SBUF relu on an early piece + a late piece; DVE big early block.
# DVE relu sub-ranges issued in a slanted (b, range) order: range r for
# batch b goes out on wave b+r, so early batches start before late chunks
# are staged and u[b] completes NR-1 waves after its first range (ring
# stays <= NR+1 deep).
DVE_RANGES = [(0, 512), (1024, 2304), (2304, 3584), (3584, 4928)]
POOL_RANGE_ALL = (512, 1024)      # Pool, all batches
POOL_RANGE_LATE = (4928, 5760)    # Pool, all batches (staged late)
ACT_RANGE = (5760, E_PAD)         # ACT, all batches, straight from PSUM

U_RING = 6                   # u[b] buffers in flight
DVE_SLOPE = 1                # range r of batch b issues on wave b + SLOPE*r

TRACE = False
LAST_RESULTS = None

_cache = {}

f32 = mybir.dt.float32
bf16 = mybir.dt.bfloat16


def _build_nc():
    nc = bacc.Bacc("TRN2", target_bir_lowering=False, debug=False)
    AF = mybir.ActivationFunctionType
    OP = mybir.AluOpType

    entT = nc.dram_tensor("entT", [D, E_PAD], bf16, kind="ExternalInput")
    w1ts = nc.dram_tensor("w1ts", [D, H], bf16, kind="ExternalInput")
    qb = nc.dram_tensor("qb", [128, 16], f32, kind="ExternalInput")
    w2c = nc.dram_tensor("w2c", [128, 2], bf16, kind="ExternalInput")
    scores_out = nc.dram_tensor("scores_out", [128, NT * B], f32,
                                kind="ExternalOutput")

    with tile.TileContext(nc) as tc:
        with (
            tc.tile_pool(name="consts", bufs=1) as consts,
            tc.tile_pool(name="ent", bufs=1) as entp,
            tc.tile_pool(name="ptsb", bufs=1) as ptsbp,
            tc.tile_pool(name="u", bufs=U_RING) as up,
            tc.tile_pool(name="sc", bufs=1) as scp,
            tc.tile_pool(name="pt_ps", bufs=2, space=bass.MemorySpace.PSUM) as pt_ps,
            tc.tile_pool(name="sc_ps", bufs=1, space=bass.MemorySpace.PSUM) as sc_ps,
        ):
            # ---- consts (Pool DMA queue; w1ts first — GEMM needs it) ----
            w1ts_sb = consts.tile([D, H], bf16, tag="w1ts")
            nc.gpsimd.dma_start(w1ts_sb[:], w1ts[:])
            qb_sb = consts.tile([128, 16], f32, tag="qb")
            nc.gpsimd.dma_start(qb_sb[:], qb[:])
            w2c_sb = consts.tile([128, 2], bf16, tag="w2c")
            nc.gpsimd.dma_start(w2c_sb[:], w2c[:])

            # ---- entity shard in 6 DMAs on the sync queue ----
            ent_sb = entp.tile([D, E_PAD], bf16, tag="ent")
            ent_cuts = [0, 512, 1536, 2560, 3584, 4608, 5632, E_PAD]
            for lo, hi in zip(ent_cuts[:-1], ent_cuts[1:]):
                nc.sync.dma_start(ent_sb[:, lo:hi], entT[:, lo:hi])

            # scores PSUM bank, also used as warmup scratch before scoring
            sc_psum_full = sc_ps.tile([128, 512], f32, tag="scps")
            sc_psum = sc_psum_full[:, 0:NT * B]

            # ---- PE warmup: dummy matmuls to lift the p-state while the
            # entity DMAs run (results discarded / overwritten later)
            junk_sb = scp.tile([128, 512], bf16, tag="junk")
            nc.vector.memset(junk_sb[:], 0.0)
            for _ in range(2):
                nc.tensor.matmul(sc_psum_full[:], junk_sb[:, 0:128],
                                 junk_sb[:], start=True, stop=True)

            # ---- GEMM chunks + ACT staging to bf16 SBUF; the last chunk
            # is never staged — its psum tile stays pinned for ACT's relu
            pt_sb = ptsbp.tile([128, 2, E_PAD], bf16, tag="pt")
            tail_ps = None
            for c in range(NCHUNK):
                w = CHUNK_W[c]
                o = CHUNK_OFF[c]
                ps = pt_ps.tile([128, 1024], f32, tag="ptps", name="pt_ps")
                for h in range(2):
                    nc.tensor.matmul(
                        ps[:, h * 512:h * 512 + w],
                        w1ts_sb[:, h * 128:(h + 1) * 128],
                        ent_sb[:, o:o + w],
                        start=True, stop=True,
                    )
                if c == NCHUNK - 1:
                    tail_ps = ps
                    continue
                src = ps[:].rearrange("p (h w) -> p h w", h=2)[:, :, 0:w]
                dst = pt_sb[:, :, o:o + w]
                nc.scalar.activation(dst, src, AF.Copy)

            # ---- relu planes: u[b][h, half, e] = relu(pt + qb[b,h]) ----
            u_tiles = [up.tile([128, 2, E_PAD], bf16, tag="u", name=f"u{b}")
                       for b in range(B)]

            def bias_ap(b, h):
                return qb_sb[:, h * 8 + b:h * 8 + b + 1]

            # DVE: slanted (wave = b + range) issue order, then late-batch
            # coverage of the ACT/Pool tail columns for b in {6, 7}
            NR = len(DVE_RANGES)
            for wave in range(B + DVE_SLOPE * (NR - 1)):
                for r in range(NR):
                    b = wave - DVE_SLOPE * r
                    if not (0 <= b < B):
                        continue
                    lo, hi = DVE_RANGES[r]
                    for h in range(2):
                        nc.vector.tensor_scalar(
                            u_tiles[b][:, h, lo:hi], pt_sb[:, h, lo:hi],
                            bias_ap(b, h), 0.0, OP.add, OP.max)
            # ACT: relu its tail chunk straight from the pinned PSUM tile
            sc_sb = scp.tile([128, NT * B], f32, tag="scsb")
            evac_cuts = [0, 4 * NT, 6 * NT, 8 * NT]
            lo, hi = ACT_RANGE
            for b in range(B):
                for h in range(2):
                    nc.scalar.activation(
                        u_tiles[b][:, h, lo:hi],
                        tail_ps[:, h * 512:h * 512 + (hi - lo)],
                        AF.Relu, bias=bias_ap(b, h), scale=1.0)
            for lo, hi, nb in (POOL_RANGE_ALL + (B,), POOL_RANGE_LATE + (B,)):
                for b in range(nb):
                    for h in range(2):
                        nc.gpsimd.tensor_scalar(
                            u_tiles[b][:, h, lo:hi], pt_sb[:, h, lo:hi],
                            bias_ap(b, h), 0.0, OP.add, OP.max)
            # ---- reduce: scores[e-tile t, col b*NT+t] = sum_h w2[h] u[b,h,e]
            for b in range(B):
                ub = u_tiles[b]
                for t in range(NT):
                    col = b * NT + t
                    for h in range(2):
                        nc.tensor.matmul(
                            sc_psum[:, col:col + 1],
                            ub[:, h, t * 128:(t + 1) * 128],
                            w2c_sb[:, h:h + 1],
                            start=(b == 0 and t == 0 and h == 0),
                            stop=(b == B - 1 and t == NT - 1 and h == 1),
                            skip_group_check=True,
                        )
                if b in (3, 5):
                    i = 0 if b == 3 else 1
                    lo, hi = evac_cuts[i], evac_cuts[i + 1]
                    nc.scalar.activation(sc_sb[:, lo:hi], sc_psum[:, lo:hi],
                                         AF.Copy)
                    nc.sync.dma_start(scores_out[:, lo:hi], sc_sb[:, lo:hi])

            # ---- evacuate the final score piece ----
            lo, hi = evac_cuts[2], evac_cuts[3]
            nc.scalar.activation(sc_sb[:, lo:hi], sc_psum[:, lo:hi], AF.Copy)
            nc.sync.dma_start(scores_out[:, lo:hi], sc_sb[:, lo:hi])

    nc.compile()
    return nc


def host_prep(head, relation, ent_emb, rel_emb, W1, b1, W2):
    """Build per-core inputs: transposed bf16 entity shards, bf16 W1t,
    fp32 qb (=ph) in [h-part, half*8+b] layout, bf16 W2 columns."""
    W1h, W1r, W1t = W1[:D], W1[D:2 * D], W1[2 * D:]
    ph = (ent_emb[head] @ W1h + rel_emb[relation] @ W1r + b1).astype(np.float32)

    w1ts_np = np.ascontiguousarray(W1t).astype(ml_dtypes.bfloat16)  # [D, H]
    qb_np = np.ascontiguousarray(
        ph.T.reshape(2, 128, B).transpose(1, 0, 2).reshape(128, 16)
    ).astype(np.float32)
    w2c_np = np.ascontiguousarray(
        W2.reshape(2, 128).T).astype(ml_dtypes.bfloat16)            # [128, 2]

    shards = []
    for c in range(NCORES):
        shT = np.zeros((D, E_PAD), dtype=ml_dtypes.bfloat16)
        shT[:, :E_LOC] = ent_emb[c * E_LOC:(c + 1) * E_LOC].T.astype(
            ml_dtypes.bfloat16)
        shards.append(shT)
    return w1ts_np, qb_np, w2c_np, shards, ph


def kernel(head, relation, k, ent_emb, rel_emb, W1, b1, W2, b2):
    head = np.asarray(head)
    relation = np.asarray(relation)
    k = int(k)
    ent_emb = np.asarray(ent_emb, dtype=np.float32)
    rel_emb = np.asarray(rel_emb, dtype=np.float32)
    W1 = np.asarray(W1, dtype=np.float32)
    b1 = np.asarray(b1, dtype=np.float32)
    W2 = np.asarray(W2, dtype=np.float32)
    b2 = np.asarray(b2, dtype=np.float32)

    w1ts_np, qb_np, w2c_np, shards, ph = host_prep(
        head, relation, ent_emb, rel_emb, W1, b1, W2)

    if "nc" not in _cache:
        _cache["nc"] = _build_nc()
    nc = _cache["nc"]

    in_maps = [
        {"entT": shards[c], "w1ts": w1ts_np, "qb": qb_np, "w2c": w2c_np}
        for c in range(NCORES)
    ]
    res = bass_utils.run_bass_kernel_spmd(
        nc, in_maps, core_ids=list(range(NCORES)), trace=TRACE)
    global LAST_RESULTS
    LAST_RESULTS = res

    # ---- host: reassemble approx scores [B, E] ----
    # scores_out[c][p, b*NT+t] = approx score(b, e = c*E_LOC + t*128 + p)
    sc = np.stack([np.asarray(r["scores_out"], np.float32)
                   for r in res.results])            # [C, 128, B*NT]
    sc = sc.reshape(NCORES, 128, B, NT).transpose(0, 2, 3, 1)  # [C, b, t, p]
    sc = sc.reshape(NCORES, B, E_PAD)[:, :, :E_LOC]            # [C, b, e_loc]
    approx = sc.transpose(1, 0, 2).reshape(B, E)               # [B, E]

    # ---- host: exact rescore of top candidates ----
    C_CAND = 256
    assert k <= C_CAND
    W1t = W1[2 * D:]
    top_indices = np.empty((B, k), np.int32)
    top_scores = np.empty((B, k), np.float32)
    for b in range(B):
        cand = np.argpartition(-approx[b], C_CAND - 1)[:C_CAND]
        x = ent_emb[cand] @ W1t + ph[b]              # [C_CAND, H] fp32
        ex = np.maximum(x, 0.0) @ W2 + b2[0]         # [C_CAND]
        order = np.lexsort((cand, -ex))[:k]
        top_indices[b] = cand[order]
        top_scores[b] = ex[order]

    return top_indices, top_scores
